# revision 3
# baseline (speedup 1.0000x reference)
"""Trainium2 Bass kernel for nn_CorticalColumn (topk_masking, 8 cores).

Reference op:
    gate = x @ Wg + bg                      # [N]
    idx  = top_k(gate, K=1638)
    act  = relu(x[idx] @ W1 + b1) @ W2 + b2 # [K, DIM]
    out  = zeros_like(x).at[idx].set(act);  mask = zeros(N).at[idx].set(1)

Strategy (8 NeuronCores, full inputs in / full output out):
  Phase A (device, data-parallel): shard x row-wise, 4096 rows/core.
    Per tile of 128 rows: DVE elementwise x*Wg, ACT accumulate-reduce
    along the free dim -> per-row gate scores.  DMA-bound (32 MB/core).
  Host: exact global top-k over the 32768 fp32 scores (tiny; boundary gap
    for this problem is ~1.6e-4 >> fp32 noise ~1e-6, so the selected SET
    matches any fp32 evaluation, incl. the reference's).
  Phase B (device, data-parallel): the K selected rows are split
    contiguously across cores (205/204 each), padded to a static M.
    Activations are kept transposed ([DIM, M]: contraction on
    partitions) so no on-device transposes are needed:
        hT = relu(W1.T @ xaT + b1);  outT = W2.T @ hT + b2
    Weights stream through SBUF in [128,16,128] panels; 16 PSUM-
    accumulated matmuls per output block.
  Host: scatter the compact results into the zero output + build mask.

MM_MODE selects matmul precision:
  "f32"  - exact fp32 matmuls (4 cycles/row on PE), rel err ~2e-7
  "f32r" - fp32r (TF32-like) matmuls at full PE rate, rel err ~2e-4
"""

import numpy as np

import concourse.bacc as bacc
import concourse.mybir as mybir
import concourse.tile as tile

N = 32768
DIM = 2048
K = 1638
P = 128
NCORES = 8
ROWS_PER_CORE = N // NCORES          # 4096
T_GATE = ROWS_PER_CORE // P          # 32 tiles of 128 rows
KO = DIM // P                        # 16 contraction blocks

MM_MODE = "f32r"                     # "f32" or "f32r"
M_PAD = 256 if MM_MODE == "f32r" else 208

F32 = mybir.dt.float32
F32R = mybir.dt.float32r

_NC_CACHE: dict = {}


def build_gate_nc(repeat: int = 1):
    """Per-core: scores[p, t] = sum_d x[t*128+p, d] * wg[d]."""
    nc = bacc.Bacc("TRN2", target_bir_lowering=False)
    x = nc.dram_tensor("x", [ROWS_PER_CORE, DIM], F32, kind="ExternalInput")
    wg = nc.dram_tensor("wg", [DIM], F32, kind="ExternalInput")
    scores = nc.dram_tensor("scores", [P, T_GATE], F32, kind="ExternalOutput")
    xt = x.rearrange("(t p) d -> t p d", p=P)

    with tile.TileContext(nc) as tc:
        with (
            tc.tile_pool(name="const", bufs=1) as const,
            tc.tile_pool(name="xp", bufs=4) as xp,
            tc.tile_pool(name="scratch", bufs=4) as scratch,
            tc.tile_pool(name="outp", bufs=1) as outp,
        ):
            wg_row = const.tile([1, DIM], F32)
            nc.sync.dma_start(wg_row[:1, :], wg[None, :])
            wg_sb = const.tile([P, DIM], F32)
            nc.gpsimd.partition_broadcast(wg_sb[:], wg_row[:1, :])
            sc_sb = outp.tile([P, T_GATE], F32)
            for _ in range(repeat):
                for t in range(T_GATE):
                    x_sb = xp.tile([P, DIM], F32, tag="x")
                    nc.sync.dma_start(x_sb[:], xt[t])
                    prod = scratch.tile([P, DIM], F32, tag="prod")
                    nc.vector.tensor_tensor(
                        prod[:], x_sb[:], wg_sb[:], mybir.AluOpType.mult
                    )
                    scr = scratch.tile([P, DIM], F32, tag="scr")
                    nc.scalar.activation(
                        scr[:], prod[:],
                        mybir.ActivationFunctionType.Copy,
                        accum_out=sc_sb[:, t : t + 1],
                    )
            nc.sync.dma_start(scores[:], sc_sb[:])
    nc.compile()
    return nc


def build_mlp_nc(repeat: int = 1):
    """Per-core 2-layer MLP on transposed activations.

    outT = W2.T @ relu(W1.T @ xaT + b1) + b2    (all [DIM, M] column-major rows)
    """
    M = M_PAD
    use_r = MM_MODE == "f32r"
    nc = bacc.Bacc("TRN2", target_bir_lowering=False)
    xaT = nc.dram_tensor("xaT", [DIM, M], F32, kind="ExternalInput")
    W1 = nc.dram_tensor("W1", [DIM, DIM], F32, kind="ExternalInput")
    b1 = nc.dram_tensor("b1", [DIM], F32, kind="ExternalInput")
    W2 = nc.dram_tensor("W2", [DIM, DIM], F32, kind="ExternalInput")
    b2 = nc.dram_tensor("b2", [DIM], F32, kind="ExternalInput")
    outT = nc.dram_tensor("outT", [DIM, M], F32, kind="ExternalOutput")

    xaT_v = xaT.rearrange("(ko p) m -> p ko m", p=P)
    W1_v = W1.rearrange("(ko p) i -> p ko i", p=P)
    W2_v = W2.rearrange("(ko p) i -> p ko i", p=P)
    b1_v = b1.rearrange("(io p) -> p io", p=P)
    b2_v = b2.rearrange("(io p) -> p io", p=P)
    outT_v = outT.rearrange("(io p) m -> p io m", p=P)

    with tile.TileContext(nc) as tc:
        with (
            tc.tile_pool(name="acts", bufs=1) as acts,
            tc.tile_pool(name="wpool", bufs=4) as wpool,
            tc.tile_pool(name="wrpool", bufs=4) as wrpool,
            tc.tile_pool(name="psum", bufs=8, space="PSUM") as psum,
            tc.tile_pool(name="outp", bufs=4) as outp,
            tc.tile_pool(name="const", bufs=1) as const,
        ):
            b1_sb = const.tile([P, KO], F32)
            nc.sync.dma_start(b1_sb[:], b1_v)
            b2_sb = const.tile([P, KO], F32)
            nc.sync.dma_start(b2_sb[:], b2_v)
            x_sb = acts.tile([P, KO, M], F32)
            for q in range(4):
                nc.sync.dma_start(
                    x_sb[:, q * 4 : (q + 1) * 4, :],
                    xaT_v[:, q * 4 : (q + 1) * 4, :],
                )
            if use_r:
                xr_sb = acts.tile([P, KO, M], F32R)
                nc.vector.tensor_copy(xr_sb[:], x_sb[:])
                h_sb = acts.tile([P, KO, M], F32R)
            else:
                xr_sb = x_sb
                h_sb = acts.tile([P, KO, M], F32)

            for _ in range(repeat):
                for layer, (W_v, rhs_sb) in enumerate(
                    [(W1_v, xr_sb), (W2_v, h_sb)]
                ):
                    for io in range(KO):
                        w_sb = wpool.tile([P, KO, P], F32, tag="w")
                        nc.sync.dma_start(
                            w_sb[:], W_v[:, :, io * P : (io + 1) * P]
                        )
                        if use_r:
                            wmm = wrpool.tile([P, KO, P], F32R, tag="wr")
                            nc.vector.tensor_copy(wmm[:], w_sb[:])
                        else:
                            wmm = w_sb
                        ps = psum.tile([P, M], F32)
                        for ko in range(KO):
                            nc.tensor.matmul(
                                ps[:],
                                lhsT=wmm[:, ko, :],
                                rhs=rhs_sb[:, ko, :],
                                start=(ko == 0),
                                stop=(ko == KO - 1),
                            )
                        if layer == 0:
                            nc.scalar.activation(
                                h_sb[:, io, :], ps[:],
                                mybir.ActivationFunctionType.Relu,
                                bias=b1_sb[:, io : io + 1],
                            )
                        else:
                            o_sb = outp.tile([P, M], F32, tag="o")
                            nc.scalar.activation(
                                o_sb[:], ps[:],
                                mybir.ActivationFunctionType.Identity,
                                bias=b2_sb[:, io : io + 1],
                            )
                            nc.sync.dma_start(outT_v[:, io, :], o_sb[:])
    nc.compile()
    return nc


def _get_nc(which: str, repeat: int = 1):
    key = (which, repeat, MM_MODE)
    if key not in _NC_CACHE:
        _NC_CACHE[key] = (
            build_gate_nc(repeat) if which == "gate" else build_mlp_nc(repeat)
        )
    return _NC_CACHE[key]


def run_spmd(nc, in_maps):
    from concourse.bass_utils import run_bass_kernel_spmd

    return run_bass_kernel_spmd(nc, in_maps, core_ids=list(range(NCORES)))


def gate_scores(x: np.ndarray, wg: np.ndarray) -> np.ndarray:
    """Device phase A: full [N] gate scores (without +bg; constant shift
    does not affect top-k and the scores are not part of the output)."""
    nc = _get_nc("gate")
    shards = [
        np.ascontiguousarray(x[c * ROWS_PER_CORE : (c + 1) * ROWS_PER_CORE])
        for c in range(NCORES)
    ]
    res = run_spmd(nc, [{"x": s, "wg": wg} for s in shards])
    # scores[p, t] holds row t*128+p of the shard
    return np.concatenate(
        [np.asarray(res.results[c]["scores"]).T.ravel() for c in range(NCORES)]
    )


def kernel(x, W1, b1, W2, b2, Wg, bg):
    x = np.ascontiguousarray(np.asarray(x, dtype=np.float32))
    W1 = np.ascontiguousarray(np.asarray(W1, dtype=np.float32))
    b1 = np.ascontiguousarray(np.asarray(b1, dtype=np.float32))
    W2 = np.ascontiguousarray(np.asarray(W2, dtype=np.float32))
    b2 = np.ascontiguousarray(np.asarray(b2, dtype=np.float32))
    Wg = np.ascontiguousarray(np.asarray(Wg, dtype=np.float32))

    # ---- Phase A: gate scores on device ----
    scores = gate_scores(x, Wg)

    # ---- Host: exact global top-k (tiny) ----
    top_idx = np.argpartition(-scores, K)[:K]
    groups = np.array_split(top_idx, NCORES)   # 205/204 rows per core

    # ---- Phase B: MLP on selected rows, data-parallel ----
    nc_b = _get_nc("mlp")
    in_maps = []
    idx_pads = []
    for g in groups:
        pad = np.full(M_PAD - len(g), g[0], dtype=g.dtype)
        idx_pad = np.concatenate([g, pad])
        idx_pads.append(idx_pad)
        xaT = np.ascontiguousarray(x[idx_pad].T)      # [DIM, M_PAD]
        in_maps.append(
            {"xaT": xaT, "W1": W1, "b1": b1, "W2": W2, "b2": b2}
        )
    res = run_spmd(nc_b, in_maps)

    # ---- Host: scatter into the zero output ----
    out = np.zeros((N, DIM), dtype=np.float32)
    for c, g in enumerate(groups):
        outT = np.asarray(res.results[c]["outT"])     # [DIM, M_PAD]
        out[g] = outT.T[: len(g)]
    mask = np.zeros(N, dtype=np.float32)
    mask[top_idx] = 1.0
    return out, mask


# revision 8
# speedup vs baseline: 464.4357x; 464.4357x over previous
"""Trainium2 Bass kernel for nn_CorticalColumn (topk_masking, 8 cores).

Reference op:
    gate = x @ Wg + bg                      # [N]
    idx  = top_k(gate, K=1638)
    act  = relu(x[idx] @ W1 + b1) @ W2 + b2 # [K, DIM]
    out  = zeros_like(x).at[idx].set(act);  mask = zeros(N).at[idx].set(1)

Strategy (8 NeuronCores, full inputs in / full output out):
  Phase A (device, data-parallel): shard x row-wise, 4096 rows/core.
    Per tile of 128 rows: DVE elementwise x*Wg, ACT accumulate-reduce
    along the free dim -> per-row gate scores.  DMA-bound (32 MB/core).
  Host: exact global top-k over the 32768 fp32 scores (tiny; boundary gap
    for this problem is ~1.6e-4 >> fp32 noise ~1e-6, so the selected SET
    matches any fp32 evaluation, incl. the reference's).
  Phase B (device, data-parallel): the K selected rows are split
    contiguously across cores (205/204 each), padded to a static M.
    Activations are kept transposed ([DIM, M]: contraction on
    partitions) so no on-device transposes are needed:
        hT = relu(W1.T @ xaT + b1);  outT = W2.T @ hT + b2
    Weights stream through SBUF in [128,16,128] panels; 16 PSUM-
    accumulated matmuls per output block.
  Host: scatter the compact results into the zero output + build mask.

MM_MODE selects matmul precision:
  "f32"  - exact fp32 matmuls (4 cycles/row on PE), rel err ~2e-7
  "f32r" - fp32r (TF32-like) matmuls at full PE rate, rel err ~2e-4
"""

import numpy as np

import concourse.bacc as bacc
import concourse.mybir as mybir
import concourse.tile as tile

N = 32768
DIM = 2048
K = 1638
P = 128
NCORES = 8
ROWS_PER_CORE = N // NCORES          # 4096
T_GATE = ROWS_PER_CORE // P          # 32 tiles of 128 rows
KO = DIM // P                        # 16 contraction blocks

MM_MODE = "f32r"                     # "f32" or "f32r"
M_PAD = 256 if MM_MODE == "f32r" else 208

F32 = mybir.dt.float32
F32R = mybir.dt.float32r

_NC_CACHE: dict = {}


GATE_B = 1  # row-tiles per DMA batch


def build_gate_nc(repeat: int = 1):
    """Per-core: scores[p, t] = sum_d x[t*128+p, d] * wg[d]."""
    nc = bacc.Bacc("TRN2", target_bir_lowering=False)
    x = nc.dram_tensor("x", [ROWS_PER_CORE, DIM], F32, kind="ExternalInput")
    wg = nc.dram_tensor("wg", [DIM], F32, kind="ExternalInput")
    scores = nc.dram_tensor("scores", [P, T_GATE], F32, kind="ExternalOutput")
    xb = x.rearrange("(b t p) d -> b p t d", p=P, t=GATE_B)

    with tile.TileContext(nc) as tc:
        with (
            tc.tile_pool(name="const", bufs=1) as const,
            tc.tile_pool(name="xp", bufs=3) as xp,
            tc.tile_pool(name="scratch", bufs=3) as scratch,
            tc.tile_pool(name="outp", bufs=1) as outp,
        ):
            wg_row = const.tile([1, DIM], F32)
            nc.sync.dma_start(wg_row[:1, :], wg[None, :])
            wg_sb = const.tile([P, DIM], F32)
            nc.gpsimd.partition_broadcast(wg_sb[:], wg_row[:1, :])
            sc_sb = outp.tile([P, T_GATE], F32)
            for _ in range(repeat):
                for b in range(T_GATE // GATE_B):
                    x_sb = xp.tile([P, GATE_B, DIM], F32, tag="x")
                    nc.sync.dma_start(x_sb[:], xb[b])
                    for t in range(GATE_B):
                        prod = scratch.tile([P, DIM], F32, tag="prod")
                        nc.vector.tensor_tensor(
                            prod[:], x_sb[:, t, :], wg_sb[:],
                            mybir.AluOpType.mult,
                        )
                        scr = scratch.tile([P, DIM], F32, tag="scr")
                        nc.scalar.activation(
                            scr[:], prod[:],
                            mybir.ActivationFunctionType.Copy,
                            accum_out=sc_sb[:, b * GATE_B + t : b * GATE_B + t + 1],
                        )
            nc.sync.dma_start(scores[:], sc_sb[:])
    nc.compile()
    return nc


def build_mlp_nc(repeat: int = 1):
    """Per-core 2-layer MLP on transposed activations.

    outT = W2.T @ relu(W1.T @ xaT + b1) + b2    (all [DIM, M] column-major rows)
    """
    M = M_PAD
    use_r = MM_MODE == "f32r"
    nc = bacc.Bacc("TRN2", target_bir_lowering=False)
    xaT = nc.dram_tensor("xaT", [DIM, M], F32, kind="ExternalInput")
    W1 = nc.dram_tensor("W1", [DIM, DIM], F32, kind="ExternalInput")
    b1 = nc.dram_tensor("b1", [DIM], F32, kind="ExternalInput")
    W2 = nc.dram_tensor("W2", [DIM, DIM], F32, kind="ExternalInput")
    b2 = nc.dram_tensor("b2", [DIM], F32, kind="ExternalInput")
    outT = nc.dram_tensor("outT", [DIM, M], F32, kind="ExternalOutput")

    xaT_v = xaT.rearrange("(ko p) m -> p ko m", p=P)
    W1_v = W1.rearrange("(ko p) i -> p ko i", p=P)
    W2_v = W2.rearrange("(ko p) i -> p ko i", p=P)
    b1_v = b1.rearrange("(io p) -> p io", p=P)
    b2_v = b2.rearrange("(io p) -> p io", p=P)
    outT_v = outT.rearrange("(io p) m -> p io m", p=P)

    with tile.TileContext(nc) as tc:
        with (
            tc.tile_pool(name="acts", bufs=1) as acts,
            tc.tile_pool(name="wpool", bufs=4) as wpool,
            tc.tile_pool(name="wrpool", bufs=4) as wrpool,
            tc.tile_pool(name="psum", bufs=8, space="PSUM") as psum,
            tc.tile_pool(name="outp", bufs=4) as outp,
            tc.tile_pool(name="const", bufs=1) as const,
        ):
            b1_sb = const.tile([P, KO], F32)
            nc.sync.dma_start(b1_sb[:], b1_v)
            b2_sb = const.tile([P, KO], F32)
            nc.sync.dma_start(b2_sb[:], b2_v)
            x_sb = acts.tile([P, KO, M], F32)
            if use_r:
                xr_sb = acts.tile([P, KO, M], F32R)
                h_sb = acts.tile([P, KO, M], F32R)
            else:
                xr_sb = x_sb
                h_sb = acts.tile([P, KO, M], F32)
            for q in range(8):
                sl = slice(q * 2, (q + 1) * 2)
                nc.sync.dma_start(x_sb[:, sl, :], xaT_v[:, sl, :])
                if use_r:
                    nc.vector.tensor_copy(xr_sb[:, sl, :], x_sb[:, sl, :])

            IOB = 1  # io blocks per weight-panel DMA
            for _ in range(repeat):
                for layer, (W_v, rhs_sb) in enumerate(
                    [(W1_v, xr_sb), (W2_v, h_sb)]
                ):
                    for iop in range(KO // IOB):
                        w_sb = wpool.tile([P, KO, IOB * P], F32, tag="w")
                        nc.sync.dma_start(
                            w_sb[:],
                            W_v[:, :, iop * IOB * P : (iop + 1) * IOB * P],
                        )
                        if use_r:
                            wmm = wrpool.tile([P, KO, IOB * P], F32R, tag="wr")
                            nc.vector.tensor_copy(wmm[:], w_sb[:])
                        else:
                            wmm = w_sb
                        for sub in range(IOB):
                            io = iop * IOB + sub
                            ps = psum.tile([P, M], F32)
                            for ko in range(KO):
                                nc.tensor.matmul(
                                    ps[:],
                                    lhsT=wmm[:, ko, sub * P : (sub + 1) * P],
                                    rhs=rhs_sb[:, ko, :],
                                    start=(ko == 0),
                                    stop=(ko == KO - 1),
                                )
                            if layer == 0:
                                nc.scalar.activation(
                                    h_sb[:, io, :], ps[:],
                                    mybir.ActivationFunctionType.Relu,
                                    bias=b1_sb[:, io : io + 1],
                                )
                            else:
                                o_sb = outp.tile([P, M], F32, tag="o")
                                nc.scalar.activation(
                                    o_sb[:], ps[:],
                                    mybir.ActivationFunctionType.Identity,
                                    bias=b2_sb[:, io : io + 1],
                                )
                                nc.sync.dma_start(outT_v[:, io, :], o_sb[:])
    nc.compile()
    return nc


def _get_nc(which: str, repeat: int = 1):
    key = (which, repeat, MM_MODE)
    if key not in _NC_CACHE:
        _NC_CACHE[key] = (
            build_gate_nc(repeat) if which == "gate" else build_mlp_nc(repeat)
        )
    return _NC_CACHE[key]


def run_spmd(nc, in_maps):
    from concourse.bass_utils import run_bass_kernel_spmd

    return run_bass_kernel_spmd(nc, in_maps, core_ids=list(range(NCORES)))


def gate_scores(x: np.ndarray, wg: np.ndarray) -> np.ndarray:
    """Device phase A: full [N] gate scores (without +bg; constant shift
    does not affect top-k and the scores are not part of the output)."""
    nc = _get_nc("gate")
    shards = [
        np.ascontiguousarray(x[c * ROWS_PER_CORE : (c + 1) * ROWS_PER_CORE])
        for c in range(NCORES)
    ]
    res = run_spmd(nc, [{"x": s, "wg": wg} for s in shards])
    # scores[p, t] holds row t*128+p of the shard
    return np.concatenate(
        [np.asarray(res.results[c]["scores"]).T.ravel() for c in range(NCORES)]
    )


def kernel(x, W1, b1, W2, b2, Wg, bg):
    x = np.ascontiguousarray(np.asarray(x, dtype=np.float32))
    W1 = np.ascontiguousarray(np.asarray(W1, dtype=np.float32))
    b1 = np.ascontiguousarray(np.asarray(b1, dtype=np.float32))
    W2 = np.ascontiguousarray(np.asarray(W2, dtype=np.float32))
    b2 = np.ascontiguousarray(np.asarray(b2, dtype=np.float32))
    Wg = np.ascontiguousarray(np.asarray(Wg, dtype=np.float32))

    # ---- Phase A: gate scores on device ----
    scores = gate_scores(x, Wg)

    # ---- Host: exact global top-k (tiny) ----
    top_idx = np.argpartition(-scores, K)[:K]
    groups = np.array_split(top_idx, NCORES)   # 205/204 rows per core

    # ---- Phase B: MLP on selected rows, data-parallel ----
    nc_b = _get_nc("mlp")
    in_maps = []
    idx_pads = []
    for g in groups:
        pad = np.full(M_PAD - len(g), g[0], dtype=g.dtype)
        idx_pad = np.concatenate([g, pad])
        idx_pads.append(idx_pad)
        xaT = np.ascontiguousarray(x[idx_pad].T)      # [DIM, M_PAD]
        in_maps.append(
            {"xaT": xaT, "W1": W1, "b1": b1, "W2": W2, "b2": b2}
        )
    res = run_spmd(nc_b, in_maps)

    # ---- Host: scatter into the zero output ----
    out = np.zeros((N, DIM), dtype=np.float32)
    for c, g in enumerate(groups):
        outT = np.asarray(res.results[c]["outT"])     # [DIM, M_PAD]
        out[g] = outT.T[: len(g)]
    mask = np.zeros(N, dtype=np.float32)
    mask[top_idx] = 1.0
    return out, mask
